# revision 8
# baseline (speedup 1.0000x reference)
"""Trainium2 Bass kernel for DeepGate2-style levelized-AIG GNN message passing.

Strategy (8-way SPMD on one TRN2 chip):
  - Node states hs/hf live as one bf16 DRAM tensor `hsf[N, 256]` ([hs||hf] per
    row), replicated per core, in AllGather-friendly row order.
  - Levels run sequentially.  Within a level each core computes 1/8 of the
    AND nodes and 1/8 of the NOT nodes: indirect-gather source rows, transpose
    to feature-major, run the MLP message networks + fused GRU gate matmuls on
    the tensor engine, combine gates, transpose back, then AllGather the new
    level block into every core's replica.
  - The GRU `h` input is structurally zero (each node is written exactly once
    and states start at zero), so the Whh matmuls drop out and
    out = sigmoid(-(z_pre)) * tanh(n_pre).  The third (linear) MLP layer and
    the per-node 2-edge message sum fold into the GRU gate matmuls:
    A_g = W3 @ Wih_g.T applied to the relu output of layer 2, accumulating the
    even-edge and odd-edge halves in PSUM.
"""

import numpy as np
import ml_dtypes

import concourse.bass as bass
import concourse.bacc as bacc
import concourse.tile as tile
import concourse.mybir as mybir
from concourse import bass_utils
from concourse.bass import IndirectOffsetOnAxis
from concourse.masks import make_identity

D = 128
NCORES = 8
BF = mybir.dt.bfloat16
F32 = mybir.dt.float32
I32 = mybir.dt.int32
AF = mybir.ActivationFunctionType
BF_NP = ml_dtypes.bfloat16


def _ceil128(x):
    return (x + 127) // 128 * 128


def _chunks(width, step=512):
    out = []
    s = 0
    while s < width:
        out.append((s, min(step, width - s)))
        s += step
    return out


class _Cfg:
    def __init__(self, P, K_and, K_not, L, ncores):
        assert P % 128 == 0 and K_and % ncores == 0 and K_not % ncores == 0
        self.P, self.K_and, self.K_not, self.L, self.ncores = P, K_and, K_not, L, ncores
        self.K = K_and + K_not
        self.apc = K_and // ncores          # AND nodes per core per level
        self.npcn = K_not // ncores         # NOT nodes per core per level
        self.Wa = _ceil128(self.apc)        # padded AND width (cols per half)
        self.Wn = _ceil128(self.npcn)       # padded NOT width
        self.npc = self.Wa + self.Wn        # rows per core per level (padded)
        self.E_pad = 2 * self.Wa + self.Wn  # gather columns per level
        self.NT = self.E_pad // 128         # gather tiles per level
        self.NLV = L - 1                    # computed levels
        self.NH = P + self.NLV * ncores * self.npc  # hsf rows (padded layout)
        self.N = P + self.NLV * self.K      # true node count

    def key(self):
        return (self.P, self.K_and, self.K_not, self.L, self.ncores)

    # ---- hsf row layout: [PI rows][level blocks in AllGather rank order] ----
    def hsf_rows_of_nodes(self, n):
        """Vectorized map: global node id -> padded hsf row."""
        n = np.asarray(n, dtype=np.int64)
        out = np.empty_like(n)
        pi = n < self.P
        out[pi] = n[pi]
        j = n[~pi] - self.P
        lev = j // self.K                  # 0-based level block
        r = j % self.K
        is_and = r < self.K_and
        c = np.where(is_and, r // self.apc, (r - self.K_and) // self.npcn)
        t = np.where(is_and, r % self.apc, (r - self.K_and) % self.npcn)
        off = np.where(is_and, t, self.Wa + t)
        out[~pi] = self.P + lev * self.ncores * self.npc + c * self.npc + off
        return out.astype(np.int32)


# --------------------------------------------------------------------------
# Device program builder
# --------------------------------------------------------------------------

# weight stack indices (each entry is a [128,128] bf16 lhsT matrix)
#   per branch: L1 (af has two: hs-part, hf-part), L2, Az, An
_W_AS = dict(l1=[0], l2=1, az=2, an=3)
_W_AF = dict(l1=[4, 5], l2=6, az=7, an=8)
_W_NS = dict(l1=[9], l2=10, az=11, an=12)
_W_NF = dict(l1=[13], l2=14, az=15, an=16)
NW = 17
# bias stack indices (per branch: b1, b2, -cz, cn)
_B_AS, _B_AF, _B_NS, _B_NF = (0, 1, 2, 3), (4, 5, 6, 7), (8, 9, 10, 11), (12, 13, 14, 15)
NB = 16


def build_nc(cfg, repeat=1, multi_gather=False, dma_transpose=True):
    # multi_gather=True (one indirect DMA with a [128, NT] index tile) works in
    # CoreSim but returns garbage on hardware — keep the per-tile gathers.
    c = cfg
    nc = bacc.Bacc(
        "TRN2", target_bir_lowering=False, debug=False, num_devices=c.ncores
    )
    wts = nc.dram_tensor("wts", [NW, D, D], BF, kind="ExternalInput").ap()
    bia = nc.dram_tensor("bia", [NB, D], F32, kind="ExternalInput").ap()
    idx = nc.dram_tensor(
        "idx", [max(c.NLV, 1), D, c.NT], I32, kind="ExternalInput"
    ).ap()
    piblk = nc.dram_tensor("piblk", [D, 2 * D], BF, kind="ExternalInput").ap()
    out_mine = nc.dram_tensor(
        "out_mine", [c.NLV * c.npc, 2 * D], BF, kind="ExternalOutput"
    ).ap()

    with tile.TileContext(nc) as tc:
        with (
            tc.tile_pool(name="const", bufs=1) as cpool,
            tc.tile_pool(name="sb", bufs=2) as sb,
            tc.tile_pool(name="gat", bufs=2) as gat,
            tc.tile_pool(name="mps", bufs=2, space="PSUM") as mps,
            tc.tile_pool(name="gps", bufs=4, space="PSUM") as gps,
            tc.tile_pool(name="tps", bufs=2, space="PSUM") as tps,
            tc.tile_pool(name="dram", bufs=2, space="DRAM") as dpool,
            tc.tile_pool(name="hsfpool", bufs=1, space="DRAM") as hsfpool,
            tc.tile_pool(name="dramsh", bufs=2, space="DRAM") as dsh,
        ):
            # consolidated per-core replica of all node states (local memory;
            # the per-level AllGather lands in a Shared tile which is then
            # copied into this).  Tile requires Shared tensors to have exactly
            # one writing instruction, hence the split.
            hsf = hsfpool.tile([c.NH, 2 * D], BF, name="hsf")
            ag_space = "Shared" if c.ncores > 4 else "Local"
            wsb = cpool.tile([D, NW * D], BF, name="wsb")
            nc.sync.dma_start(
                wsb[:].rearrange("p (w f) -> p w f", w=NW), wts.transpose([1, 0, 2])
            )
            bsb = cpool.tile([D, NB], F32, name="bsb")
            nc.sync.dma_start(bsb[:], bia.rearrange("b p -> p b"))
            ident = cpool.tile([D, D], BF, name="ident")
            make_identity(nc, ident[:])
            pisb = cpool.tile([D, 2 * D], BF, name="pisb")
            nc.sync.dma_start(pisb[:], piblk[:])

            def W(i):
                return wsb[:, i * D:(i + 1) * D]

            def B(i):
                return bsb[:, i:i + 1]

            for _rep in range(repeat):
                # ---- PI region init (single broadcast DMA: one writer inst) ----
                nc.sync.dma_start(
                    hsf[0:c.P, :].rearrange("(r p) f -> p r f", p=128),
                    pisb[:, None, :].to_broadcast([128, c.P // 128, 2 * D]),
                )

                # ---- levels ----
                for l in range(1, c.NLV + 1):
                    base = c.P + (l - 1) * c.ncores * c.npc

                    idxt = sb.tile([D, c.NT], I32, tag="idxt")
                    nc.sync.dma_start(idxt[:], idx[l - 1])

                    gall = gat.tile([D, c.NT, 2 * D], BF, tag="gall")
                    if multi_gather:
                        nc.gpsimd.indirect_dma_start(
                            out=gall[:, :, :],
                            out_offset=None,
                            in_=hsf[:, :],
                            in_offset=IndirectOffsetOnAxis(ap=idxt[:, :], axis=0),
                        )
                    else:
                        for j in range(c.NT):
                            nc.gpsimd.indirect_dma_start(
                                out=gall[:, j, :],
                                out_offset=None,
                                in_=hsf[:, :],
                                in_offset=IndirectOffsetOnAxis(
                                    ap=idxt[:, j:j + 1], axis=0
                                ),
                            )

                    xhsT = sb.tile([D, c.E_pad], BF, tag="xhsT")
                    xhfT = sb.tile([D, c.E_pad], BF, tag="xhfT")
                    for j in range(c.NT):
                        cs = slice(j * 128, (j + 1) * 128)
                        if dma_transpose:
                            nc.sync.dma_start_transpose(xhsT[:, cs], gall[:, j, 0:D])
                            nc.sync.dma_start_transpose(
                                xhfT[:, cs], gall[:, j, D:2 * D]
                            )
                        else:
                            pt1 = tps.tile([D, D], BF, tag="tpsi")
                            nc.tensor.transpose(pt1[:], gall[:, j, 0:D], ident[:])
                            nc.vector.tensor_copy(xhsT[:, cs], pt1[:])
                            pt2 = tps.tile([D, D], BF, tag="tpsi")
                            nc.tensor.transpose(pt2[:], gall[:, j, D:2 * D], ident[:])
                            nc.vector.tensor_copy(xhfT[:, cs], pt2[:])

                    def branch(tag, xins, wd, bd, col0, Wtot, and_branch):
                        """MLP L1,L2 + fused gate matmuls + combine.

                        xins: list of feature-major inputs for L1 (accumulated)
                        Returns combined output tile [D, Wtot//2 or Wtot].
                        """
                        h1 = sb.tile([D, Wtot], BF, tag=f"h1{tag}", name=f"h1{tag}")
                        for (s, sz) in _chunks(Wtot):
                            p1 = mps.tile([D, 512], F32, tag="mlp_ps", name="p1")
                            nk = len(xins)
                            for k, (xin, w1i) in enumerate(zip(xins, wd["l1"])):
                                nc.tensor.matmul(
                                    p1[:, :sz],
                                    W(w1i),
                                    xin[:, col0 + s:col0 + s + sz],
                                    start=(k == 0),
                                    stop=(k == nk - 1),
                                )
                            nc.scalar.activation(
                                h1[:, s:s + sz], p1[:, :sz], AF.Relu, bias=B(bd[0])
                            )
                        h2 = sb.tile([D, Wtot], BF, tag=f"h2{tag}", name=f"h2{tag}")
                        for (s, sz) in _chunks(Wtot):
                            p2 = mps.tile([D, 512], F32, tag="mlp_ps", name="p2")
                            nc.tensor.matmul(
                                p2[:, :sz], W(wd["l2"]), h1[:, s:s + sz],
                                start=True, stop=True,
                            )
                            nc.scalar.activation(
                                h2[:, s:s + sz], p2[:, :sz], AF.Relu, bias=B(bd[1])
                            )
                        Wg = Wtot // 2 if and_branch else Wtot
                        zp = sb.tile([D, Wg], BF, tag=f"zp{tag}", name=f"zp{tag}")
                        nn = sb.tile([D, Wg], BF, tag=f"nn{tag}", name=f"nn{tag}")
                        for (s, sz) in _chunks(Wg):
                            for gate, wi, bi, outt, func, scale in (
                                ("z", wd["az"], bd[2], zp, AF.Sigmoid, -1.0),
                                ("n", wd["an"], bd[3], nn, AF.Tanh, 1.0),
                            ):
                                pg = gps.tile(
                                    [D, 512], F32, tag="gate_ps", name="pg"
                                )
                                if and_branch:
                                    nc.tensor.matmul(
                                        pg[:, :sz], W(wi), h2[:, s:s + sz],
                                        start=True, stop=False,
                                    )
                                    nc.tensor.matmul(
                                        pg[:, :sz], W(wi), h2[:, Wg + s:Wg + s + sz],
                                        start=False, stop=True,
                                    )
                                else:
                                    nc.tensor.matmul(
                                        pg[:, :sz], W(wi), h2[:, s:s + sz],
                                        start=True, stop=True,
                                    )
                                nc.scalar.activation(
                                    outt[:, s:s + sz], pg[:, :sz], func,
                                    bias=B(bi), scale=scale,
                                )
                        cb = sb.tile([D, Wg], BF, tag=f"cb{tag}", name=f"cb{tag}")
                        nc.vector.tensor_mul(cb[:], zp[:], nn[:])
                        return cb

                    hs_and = branch("as", [xhsT], _W_AS, _B_AS, 0, 2 * c.Wa, True)
                    hf_and = branch(
                        "af", [xhsT, xhfT], _W_AF, _B_AF, 0, 2 * c.Wa, True
                    )
                    hs_not = branch("ns", [xhsT], _W_NS, _B_NS, 2 * c.Wa, c.Wn, False)
                    hf_not = branch("nf", [xhfT], _W_NF, _B_NF, 2 * c.Wa, c.Wn, False)

                    # ---- transpose back to node-major + store ----
                    bounce = dpool.tile([c.npc, 2 * D], BF, tag="bounce", name="bounce")
                    for (src, colh, row0, width) in (
                        (hs_and, 0, 0, c.Wa),
                        (hf_and, D, 0, c.Wa),
                        (hs_not, 0, c.Wa, c.Wn),
                        (hf_not, D, c.Wa, c.Wn),
                    ):
                        for t in range(width // 128):
                            if dma_transpose:
                                nm = sb.tile([D, D], BF, tag="nmout", name="nm")
                                nc.sync.dma_start_transpose(
                                    nm[:], src[:, t * 128:(t + 1) * 128]
                                )
                            else:
                                pt = tps.tile([D, D], BF, tag="tpso", name="pt")
                                nc.tensor.transpose(
                                    pt[:], src[:, t * 128:(t + 1) * 128], ident[:]
                                )
                                nm = sb.tile([D, D], BF, tag="nmout", name="nm")
                                nc.vector.tensor_copy(nm[:], pt[:])
                            nc.sync.dma_start(
                                bounce[row0 + t * 128:row0 + (t + 1) * 128,
                                       colh:colh + D],
                                nm[:],
                            )
                    nc.sync.dma_start(
                        out_mine[(l - 1) * c.npc:l * c.npc, :], bounce[:]
                    )
                    if l < c.NLV:
                        agout = dsh.tile(
                            [c.ncores * c.npc, 2 * D], BF,
                            tag="agout", name="agout", addr_space=ag_space,
                        )
                        nc.gpsimd.collective_compute(
                            "AllGather",
                            mybir.AluOpType.bypass,
                            replica_groups=[list(range(c.ncores))],
                            ins=[bounce[:]],
                            outs=[agout[:]],
                        )
                        nc.sync.dma_start(
                            hsf[base:base + c.ncores * c.npc, :], agout[:]
                        )
    nc.compile()
    return nc


# --------------------------------------------------------------------------
# Host-side preparation
# --------------------------------------------------------------------------

def _np32(x):
    return np.asarray(x, dtype=np.float32)


def _mlp_np(p, x):
    W1, b1, W2, b2, W3, b3 = [_np32(t) for t in p]
    h = np.maximum(x @ W1 + b1, 0.0)
    h = np.maximum(h @ W2 + b2, 0.0)
    return h @ W3 + b3


def _gru_h0_np(p, x):
    """torch GRU cell with h=0."""
    Wih, Whh, bih, bhh = [_np32(t) for t in p]
    gi = x @ Wih.T + bih
    gh = bhh
    ir, iz, i_n = np.split(gi, 3, -1)
    hr, hz, h_n = np.split(np.broadcast_to(gh, gi.shape), 3, -1)
    r = 1.0 / (1.0 + np.exp(-(ir + hr)))
    z = 1.0 / (1.0 + np.exp(-(iz + hz)))
    n = np.tanh(i_n + r * h_n)
    return (1.0 - z) * n


def _prep_inputs(edge_index, p_as, p_ns, p_af, p_nf, g_as, g_af, g_ns, g_nf, cfg):
    c = cfg
    ei = np.asarray(edge_index)
    E_l = 2 * c.K_and + c.K_not

    # --- structural validation of dst pattern (we rely on it) ---
    src = ei[0].astype(np.int64)
    dst = ei[1].astype(np.int64)
    l_arr = np.arange(1, c.L)
    starts = c.P + (l_arr - 1) * c.K
    exp_dst = np.concatenate([
        np.concatenate([
            np.repeat(np.arange(s, s + c.K_and), 2),
            np.arange(s + c.K_and, s + c.K),
        ])
        for s in starts
    ])
    assert dst.shape[0] == c.NLV * E_l and np.array_equal(dst, exp_dst), (
        "edge_index dst does not match the levelized AIG structure this kernel "
        "is specialized for"
    )
    assert np.all(src < np.repeat(starts, E_l)), "source from same/higher level"

    # --- GRU structural requirement: bhh == 0 (h input is always zero) ---
    for g in (g_as, g_af, g_ns, g_nf):
        assert np.allclose(_np32(g[3]), 0.0), "nonzero bhh not supported"

    # --- weight stack ---
    def fuse(pm, gm):
        W1, b1, W2, b2, W3, b3 = [_np32(t) for t in pm]
        Wih, _, bih, _ = [_np32(t) for t in gm]
        Wz, Wn_ = Wih[D:2 * D], Wih[2 * D:3 * D]
        Az = W3 @ Wz.T
        An = W3 @ Wn_.T
        return W1, b1, W2, b2, Az, An, Wz, Wn_, b3, bih

    W1a, b1a, W2a, b2a, Aza, Ana, Wza, Wna, b3a, biha = fuse(p_as, g_as)
    W1f, b1f, W2f, b2f, Azf, Anf, Wzf, Wnf, b3f, bihf = fuse(p_af, g_af)
    W1n, b1n, W2n, b2n, Azn, Ann, Wzn, Wnn, b3n, bihn = fuse(p_ns, g_ns)
    W1m, b1m, W2m, b2m, Azm, Anm, Wzm, Wnm, b3m, bihm = fuse(p_nf, g_nf)

    wstack = np.stack([
        W1a, W2a, Aza, Ana,
        W1f[:D], W1f[D:], W2f, Azf, Anf,
        W1n, W2n, Azn, Ann,
        W1m, W2m, Azm, Anm,
    ]).astype(BF_NP)

    def gate_bias(Wg, b3, bih_g, mult):
        return Wg @ (mult * b3) + bih_g

    bstack = np.stack([
        b1a, b2a, -gate_bias(Wza, b3a, biha[D:2 * D], 2.0),
        gate_bias(Wna, b3a, biha[2 * D:], 2.0),
        b1f, b2f, -gate_bias(Wzf, b3f, bihf[D:2 * D], 2.0),
        gate_bias(Wnf, b3f, bihf[2 * D:], 2.0),
        b1n, b2n, -gate_bias(Wzn, b3n, bihn[D:2 * D], 1.0),
        gate_bias(Wnn, b3n, bihn[2 * D:], 1.0),
        b1m, b2m, -gate_bias(Wzm, b3m, bihm[D:2 * D], 1.0),
        gate_bias(Wnm, b3m, bihm[2 * D:], 1.0),
    ]).astype(np.float32)

    # --- PI init (two virtual NOT hops on zero state) ---
    z1 = np.zeros((1, D), np.float32)
    h1 = _gru_h0_np(g_nf, _mlp_np(p_nf, z1))
    h2 = _gru_h0_np(g_nf, _mlp_np(p_nf, h1))
    pirow = np.concatenate([np.zeros((1, D), np.float32), h2], axis=1)
    piblk = np.broadcast_to(pirow, (D, 2 * D)).astype(BF_NP)

    # --- per-core gather indices ---
    hrow = c.hsf_rows_of_nodes(src)  # [NLV*E_l]
    idx_all = []
    for core in range(c.ncores):
        per_lvl = np.zeros((c.NLV, c.E_pad), np.int32)
        for li in range(c.NLV):
            e0 = li * E_l
            # AND pairs for this core
            a0 = core * c.apc
            ev = hrow[e0 + 2 * a0 + 0: e0 + 2 * (a0 + c.apc): 2]
            od = hrow[e0 + 2 * a0 + 1: e0 + 2 * (a0 + c.apc): 2]
            per_lvl[li, 0:c.apc] = ev
            per_lvl[li, c.Wa:c.Wa + c.apc] = od
            # NOT edges
            n0 = e0 + 2 * c.K_and + core * c.npcn
            per_lvl[li, 2 * c.Wa:2 * c.Wa + c.npcn] = hrow[n0:n0 + c.npcn]
        # [NLV, E_pad] -> [NLV, 128, NT]  (partition-major per gather tile)
        idx_all.append(
            np.ascontiguousarray(
                per_lvl.reshape(c.NLV, c.NT, 128).transpose(0, 2, 1)
            )
        )

    in_maps = [
        {
            "wts": wstack,
            "bia": bstack,
            "idx": idx_all[core],
            "piblk": np.ascontiguousarray(piblk),
        }
        for core in range(c.ncores)
    ]
    return in_maps, h2.astype(np.float32)


def _assemble(results, cfg, h2):
    c = cfg
    hs = np.zeros((c.N, D), np.float32)
    hf = np.zeros((c.N, D), np.float32)
    hf[:c.P] = h2
    for core in range(c.ncores):
        om = np.asarray(results[core]["out_mine"]).astype(np.float32)
        blk = om.reshape(c.NLV, c.npc, 2 * D)
        for li in range(c.NLV):
            start = c.P + li * c.K
            a0 = start + core * c.apc
            hs[a0:a0 + c.apc] = blk[li, 0:c.apc, :D]
            hf[a0:a0 + c.apc] = blk[li, 0:c.apc, D:]
            n0 = start + c.K_and + core * c.npcn
            hs[n0:n0 + c.npcn] = blk[li, c.Wa:c.Wa + c.npcn, :D]
            hf[n0:n0 + c.npcn] = blk[li, c.Wa:c.Wa + c.npcn, D:]
    return hs, hf


_NC_CACHE = {}


def _get_nc(cfg, **kw):
    key = (cfg.key(), tuple(sorted(kw.items())))
    if key not in _NC_CACHE:
        _NC_CACHE[key] = build_nc(cfg, **kw)
    return _NC_CACHE[key]


def kernel(edge_index, p_as, p_ns, p_af, p_nf, g_as, g_af, g_ns, g_nf,
           P, K_and, K_not, L):
    cfg = _Cfg(int(P), int(K_and), int(K_not), int(L), NCORES)
    in_maps, h2 = _prep_inputs(
        edge_index, p_as, p_ns, p_af, p_nf, g_as, g_af, g_ns, g_nf, cfg
    )
    nc = _get_nc(cfg)
    res = bass_utils.run_bass_kernel_spmd(
        nc, in_maps, core_ids=list(range(NCORES))
    )
    return _assemble(res.results, cfg, h2)
